# revision 29
# baseline (speedup 1.0000x reference)
"""Trainium2 Bass kernel for Nadaraya-Watson kernel regression over MLP embeddings.

Computes: out[b,d] = sum_n K[n,b,d]*Y[n,d] / sum_n K[n,b,d]
where K = exp(-0.5*((z[n,d]-zw[b,d])/h)^2), z/zw are 2-layer MLP embeddings of
train/query points (ReLU MLP 64->128->10).

Strategy (8 NeuronCores, data-parallel over the train axis N, ~100us/core):
 - shard train_X/Y over 8 cores (1250 rows each, padded to 1280 = 10 tiles
   of 128); pad rows are masked via the ones-column of the Y-side weights.
 - layer-1 embedding operands are hi/lo bf16-split on the host (exact fp32
   products in the fp32 PSUM accumulator at full bf16 PE rate); layer 2 runs
   fp32. Embeddings are bit-accurate to ~1e-5.
 - u^2 = z^2 - 2 z zw + zw^2 is built as a K=7 bf16 matmul per (d, n-tile)
   from hi/lo bf16 splits of z^2, -2z, zw, zw^2 (u^2 accurate to ~1e-4),
   streamed at full PE rate; exp on ScalarE with scale=-0.5/h^2 folded in,
   over 3-PSUM-bank [128,1536] chunks (ScalarE is the ~53us bottleneck).
 - PE array packing: u^2 matmuls 2-way row-tiled (partition offsets 0/64,
   operands replicated there via one contiguous DRAM-bounce DMA per group);
   numerator+denominator reduce matmuls (M=2, contracting n on partitions)
   4-way column-tiled (t%4) into one PSUM accumulator bank per d.
 - the main loop is software-pipelined over flat uniform 3-tile chunks:
   reduce matmuls of chunk k are emitted after the u^2 matmuls + exp of
   chunk k+1 so the in-order PE queue never stalls on ScalarE.
 - each core outputs [20,512] partials (10 numerator rows, 10 denominator
   rows); the host sums the 8 partials, divides, transposes -- the
   data-parallel all-reduce + unshard step of the hint.
"""

import ml_dtypes
import numpy as np

import concourse.bass as bass
import concourse.tile as tile
from concourse import bacc, mybir
from concourse.bass_utils import run_bass_kernel_spmd

F32 = mybir.dt.float32
BF16 = mybir.dt.bfloat16
AF = mybir.ActivationFunctionType

N_CORES = 8
B = 512          # batch (queries)
DOUT = 10
DIN = 64
DHID = 128
N_TRAIN = 10000
NSH = 1280       # padded per-core shard rows
NT = NSH // 128  # 10 n-tiles per core
SHARD = N_TRAIN // N_CORES  # 1250 valid rows per core

_cache = {}


def _build(c: float):
    """Build + compile the per-core Bass kernel. c = -0.5/h^2."""
    nc = bacc.Bacc(
        "TRN2",
        target_bir_lowering=False,
        debug=False,
        enable_asserts=False,
        num_devices=N_CORES,
    )

    # layer-1 operands arrive hi/lo bf16-split from the host:
    #   xt_a/txt_a = [Xhi; Xhi] (dup at partition 64), xt_b/txt_b = Xlo
    #   w1s = [W1hiT; W1loT]
    xta_d = nc.dram_tensor("xt_a", [2 * DIN, B], BF16, kind="ExternalInput")
    xtb_d = nc.dram_tensor("xt_b", [DIN, B], BF16, kind="ExternalInput")
    txta_d = nc.dram_tensor("txt_a", [2 * DIN, NSH], BF16, kind="ExternalInput")
    txtb_d = nc.dram_tensor("txt_b", [DIN, NSH], BF16, kind="ExternalInput")
    w1s_d = nc.dram_tensor("w1s", [2 * DIN, DHID], BF16, kind="ExternalInput")
    w2t_d = nc.dram_tensor("w2t", [DHID, DOUT], F32, kind="ExternalInput")   # W2^T
    yext_d = nc.dram_tensor("yext", [NSH, 2 * DOUT], F32, kind="ExternalInput")
    out_d = nc.dram_tensor("out", [2 * DOUT, B], F32, kind="ExternalOutput")

    with tile.TileContext(nc) as tc:
        with (
            tc.tile_pool(name="const", bufs=1) as const,
            tc.tile_pool(name="work", bufs=1) as work,
            tc.tile_pool(name="sums", bufs=4) as sums,
            tc.tile_pool(name="psB", bufs=2, space=bass.MemorySpace.PSUM) as psB,
            tc.tile_pool(name="accp", bufs=2, space=bass.MemorySpace.PSUM) as accp,
            tc.tile_pool(name="ksbp", bufs=3) as ksbp,
            tc.tile_pool(name="dram", bufs=1, space="DRAM") as dram,
        ):
            # ---- load inputs (spread across DMA queues) ----
            # W1^T / trainX^T replicated at partitions 0 and 64 for 2-way
            # row-tiled K=64 embedding matmuls.
            sb_txta = const.tile([2 * DIN, NSH], BF16)
            nc.sync.dma_start(sb_txta[:], txta_d.ap())
            sb_txtb = const.tile([DIN, NSH], BF16)
            nc.sync.dma_start(sb_txtb[:], txtb_d.ap())
            sb_w1s = const.tile([2 * DIN, DHID], BF16)
            nc.scalar.dma_start(sb_w1s[:], w1s_d.ap())
            sb_w2t = const.tile([DHID, DOUT], F32)
            nc.scalar.dma_start(sb_w2t[:], w2t_d.ap())
            sb_xta = const.tile([2 * DIN, B], BF16)
            nc.scalar.dma_start(sb_xta[:], xta_d.ap())
            sb_xtb = const.tile([DIN, B], BF16)
            nc.scalar.dma_start(sb_xtb[:], xtb_d.ap())
            # yext [1280, 20] -> [128, 10, 20] (n-within-tile on partitions)
            sb_y32 = const.tile([128, NT, 2 * DOUT], F32)
            nc.gpsimd.dma_start(
                sb_y32[:], yext_d.ap().rearrange("(t p) c -> p t c", p=128)
            )
            sb_yb = const.tile([128, NT, 2 * DOUT], BF16)
            nc.vector.tensor_copy(sb_yb[:], sb_y32[:])

            # ---- embeddings ----
            # PE order: Ht -> Hq -> Zt -> Zq so the (longer) phi chain starts
            # as early as possible and the psi chain overlaps it.
            ps_ht = psB.tile([128, 1536], F32, tag="pu")
            for a, b_ in ((0, 512), (512, 1024), (1024, 1280)):
                nc.tensor.matmul(
                    ps_ht[:, a:b_], sb_w1s[:], sb_txta[:, a:b_],
                    start=True, stop=False,
                )
                nc.tensor.matmul(
                    ps_ht[:, a:b_], sb_w1s[0:DIN, :], sb_txtb[:, a:b_],
                    start=False, stop=True,
                )
            sb_ht = work.tile([DHID, NSH], F32)
            for a, b_ in ((0, 512), (512, 1024), (1024, 1280)):
                nc.scalar.activation(sb_ht[:, a:b_], ps_ht[:, a:b_], AF.Relu)

            # Zt matmuls (M=10) use col tiles 0/32/64 for 3-way concurrency.
            # sb_zt is stored group-major: n' = (g, t5, p) with tile t = 2*t5+g,
            # so the phi scratch -> replica DMAs are fully contiguous per group.
            ps_zt = psB.tile([128, 1536], F32, tag="pu")
            for i, (a, b_) in enumerate(((0, 512), (512, 1024), (1024, 1280))):
                nc.tensor.matmul(
                    ps_zt[32 * i:32 * i + DOUT, a:b_], sb_w2t[:], sb_ht[:, a:b_],
                    start=True, stop=True, tile_position=(0, 32 * i),
                )
            ps_hq = psB.tile([128, 1536], F32, tag="pu")
            nc.tensor.matmul(
                ps_hq[:, :B], sb_w1s[:], sb_xta[:], start=True, stop=False
            )
            nc.tensor.matmul(
                ps_hq[:, :B], sb_w1s[0:DIN, :], sb_xtb[:], start=False, stop=True
            )
            sb_hq = work.tile([DHID, B], F32)
            nc.vector.tensor_relu(sb_hq[:], ps_hq[:, :B])
            ps_zq = psB.tile([128, 1536], F32, tag="pu")
            nc.tensor.matmul(
                ps_zq[0:DOUT, :B], sb_w2t[:], sb_hq[:], start=True, stop=True
            )

            sb_zt2 = work.tile([DOUT, 2, 5, 128], F32)
            nc.vector.tensor_copy(
                sb_zt2[:, :, 0:2, :].rearrange("d g t p -> d t g p"),
                ps_zt[0:DOUT, 0:512].rearrange("d (t g p) -> d t g p", g=2, p=128),
            )
            nc.scalar.activation(
                sb_zt2[:, :, 2:4, :].rearrange("d g t p -> d t g p"),
                ps_zt[32:32 + DOUT, 512:1024].rearrange(
                    "d (t g p) -> d t g p", g=2, p=128
                ),
                AF.Copy,
            )
            nc.scalar.activation(
                sb_zt2[:, :, 4, :],
                ps_zt[64:64 + DOUT, 1024:1280].rearrange(
                    "d (g p) -> d g p", p=128
                ),
                AF.Copy,
            )
            sb_zt = sb_zt2[:].rearrange("d g t p -> d (g t p)")
            sb_zq = work.tile([DOUT, B], F32)
            nc.vector.tensor_copy(sb_zq[:], ps_zq[0:DOUT, :B])

            # ---- phi pieces (train side), [10, 1280] ops ----
            # u^2[n,b] = sh*1 + sl*1 + 1*qh + 1*ql + m2zh*wh + m2zh*wl + m2zl*wh
            # staging rows phi: 0 sh, 1 sl, 2 ones, 3 ones, 4 m2zh, 5 m2zh, 6 m2zl
            stg_phi = work.tile([DOUT, 7, 1280], BF16)
            # m2zh = bf16(-2 z) via ACT scale; m2zl = (-2 z) - m2zh fused on DVE
            nc.scalar.activation(stg_phi[:, 4, 0:NSH], sb_zt, AF.Copy, scale=-2.0)
            nc.vector.scalar_tensor_tensor(
                stg_phi[:, 6, 0:NSH], sb_zt, -2.0, stg_phi[:, 4, 0:NSH],
                op0=mybir.AluOpType.mult, op1=mybir.AluOpType.subtract,
            )
            # sh = bf16(z^2) on ACT in parallel with the exact fp32 square on DVE
            t_sq = work.tile([DOUT, NSH], F32)
            nc.vector.tensor_mul(t_sq[:], sb_zt, sb_zt)
            nc.scalar.activation(stg_phi[:, 0, 0:NSH], sb_zt, AF.Square)
            nc.vector.tensor_sub(stg_phi[:, 1, 0:NSH], t_sq[:], stg_phi[:, 0, 0:NSH])
            nc.gpsimd.memset(stg_phi[:, 2:4, :], 1.0)  # ones rows

            # ---- psi pieces (query side) ----
            # psi staging rows: 0 ones, 1 ones, 2 qh, 3 ql, 4 wh, 5 wl, 6 wh
            stg_psi = work.tile([DOUT, 7, B], BF16)
            nc.scalar.activation(stg_psi[:, 4, :], sb_zq[:], AF.Copy)
            nc.vector.tensor_sub(stg_psi[:, 5, :], sb_zq[:], stg_psi[:, 4, :])
            t_qw = work.tile([DOUT, B], F32)
            nc.vector.tensor_mul(t_qw[:], sb_zq[:], sb_zq[:])
            nc.scalar.activation(stg_psi[:, 2, :], sb_zq[:], AF.Square)
            nc.vector.tensor_sub(stg_psi[:, 3, :], t_qw[:], stg_psi[:, 2, :])
            nc.gpsimd.memset(stg_psi[:, 0:2, :], 1.0)

            # ---- reorder [d, r, n] -> [r, d, n] via DRAM bounce; replicate at
            # partition offsets 0/32/64/96 for row-tiled u^2 matmuls ----
            # Row-tile group of tile t is g2 = t%2 at partition offsets 0/64.
            # sb_zt's group-major n-order makes scratch and replicas fully
            # contiguous: one bounce + one replica DMA per group. Duplicated
            # rows (phi 5<-4, psi 6<-4) are materialized by the bounce DMAs.
            phic = const.tile([71, DOUT, 5, 128], BF16)
            psic = const.tile([71, DOUT, B], BF16)
            scr_phi = dram.tile([DOUT, 7, 2, 5, 128], BF16)
            scr_psi = dram.tile([7, DOUT, B], BF16)
            scr_v = scr_phi[:].rearrange("d r g t5 p -> d r (g t5 p)")
            nc.sync.dma_start(scr_v[:, 0:5, :], stg_phi[:, 0:5, :])
            nc.sync.dma_start(scr_v[:, 5:6, :], stg_phi[:, 4:5, :])
            nc.sync.dma_start(scr_v[:, 6:7, :], stg_phi[:, 6:7, :])
            psi_v = scr_psi[:].rearrange("r d b -> d r b")
            nc.scalar.dma_start(psi_v[:, 6:7, :], stg_psi[:, 4:5, :])
            nc.scalar.dma_start(psi_v[:, 0:6, :], stg_psi[:, 0:6, :])
            nc.scalar.dma_start(psic[0:7], scr_psi[:])
            nc.gpsimd.dma_start(psic[64:71], scr_psi[:])
            for g in range(2):
                eng = (nc.sync, nc.scalar)[g]
                eng.dma_start(
                    phic[64 * g:64 * g + 7],
                    scr_phi[:, :, g].rearrange("d r t5 p -> r d t5 p"),
                )

            # ---- software-pipelined main loop ----
            # flat uniform chunks of 3 tiles across the whole (d, t) list --
            # no per-d ragged chunk, so the exp pipeline never hiccups at a
            # d boundary.
            flat = [(d, t) for d in range(DOUT) for t in range(NT)]
            steps = [tuple(flat[i:i + 3]) for i in range(0, len(flat), 3)]
            res2 = work.tile([2, DOUT, B], F32)
            out_v = out_d.ap().rearrange("(two dd) b -> two dd b", two=2)
            kts = {}
            accs = {}
            for k in range(len(steps) + 1):
                if k < len(steps):
                    tt = steps[k]
                    pu = psB.tile([128, 1536], F32, tag="pu")
                    for j, (d, t) in enumerate(tt):
                        g = t % 2
                        nc.tensor.matmul(
                            pu[:, j * B:(j + 1) * B],
                            phic[64 * g:64 * g + 7, d, t // 2, :],
                            psic[64 * g:64 * g + 7, d, :],
                            start=True,
                            stop=True,
                            tile_position=(64 * g, 0),
                        )
                    kt = ksbp.tile([128, 1536], BF16, tag="kt")
                    nc.scalar.activation(
                        kt[:, : len(tt) * B], pu[:, : len(tt) * B], AF.Exp, scale=c
                    )
                    kts[k] = (kt, tt)
                if k > 0:
                    kt, tt = kts.pop(k - 1)
                    for j, (d, t) in enumerate(tt):
                        # column group gc = t%4: group gc accumulates tiles
                        # {gc, gc+4, gc+8}; distinct groups per chunk overlap.
                        if d not in accs:
                            acc_t = accp.tile([98, B], F32, tag="acc")
                            accs[d] = acc_t
                        # final d accumulates in a single group so its tail
                        # drain is one ScalarE copy instead of a DVE add chain
                        gc = 0 if d == DOUT - 1 else t % 4
                        nc.tensor.matmul(
                            accs[d][32 * gc:32 * gc + 2, :],
                            sb_yb[:, t, 2 * d:2 * d + 2],
                            kt[:, j * B:(j + 1) * B],
                            start=(t == 0 if d == DOUT - 1 else t < 4),
                            stop=(t == NT - 1 if d == DOUT - 1 else t >= NT - 4),
                            tile_position=(0, 32 * gc),
                        )
                    d9 = [d for (d, t) in tt if t == NT - 1]
                    if d9:
                        # d complete: sum the 4 column-tile partials
                        # (DVE reads at most one PSUM operand per op)
                        d = d9[0]
                        acc4 = accs.pop(d)
                        if d == DOUT - 1:
                            nc.scalar.activation(
                                res2[:, d, :], acc4[0:2, :], AF.Copy
                            )
                        else:
                            s0 = sums.tile([2, B], F32, tag="s0")
                            nc.vector.tensor_copy(s0[:], acc4[0:2, :])
                            s1 = sums.tile([2, B], F32, tag="s1")
                            nc.vector.tensor_add(s1[:], s0[:], acc4[32:34, :])
                            s2 = sums.tile([2, B], F32, tag="s2")
                            nc.vector.tensor_add(s2[:], s1[:], acc4[64:66, :])
                            nc.vector.tensor_add(
                                res2[:, d, :], s2[:], acc4[96:98, :]
                            )
                        nc.sync.dma_start(out_v[:, d, :], res2[:, d, :])

    nc.compile()
    return nc


def _split_hi_lo(a32):
    hi = a32.astype(ml_dtypes.bfloat16)
    lo = (a32 - hi.astype(np.float32)).astype(ml_dtypes.bfloat16)
    return hi, lo


def _prep_inputs(x, train_X, Y, W1, W2):
    xh, xl = _split_hi_lo(np.ascontiguousarray(x.T, dtype=np.float32))
    xt_a = np.concatenate([xh, xh], axis=0)
    w1h, w1l = _split_hi_lo(np.ascontiguousarray(W1.T, dtype=np.float32))
    w1s = np.concatenate([w1h, w1l], axis=0)
    w2t = np.ascontiguousarray(W2.T, dtype=np.float32)
    in_maps = []
    for i in range(N_CORES):
        sl = slice(i * SHARD, (i + 1) * SHARD)
        txt = np.zeros((DIN, NSH), dtype=np.float32)
        txt[:, :SHARD] = train_X[sl].T
        th, tl = _split_hi_lo(txt)
        txt_a = np.concatenate([th, th], axis=0)
        yext = np.zeros((NSH, 2 * DOUT), dtype=np.float32)
        yext[:SHARD, 0::2] = Y[sl]
        yext[:SHARD, 1::2] = 1.0
        in_maps.append(
            {"xt_a": xt_a, "xt_b": xl, "w1s": w1s, "w2t": w2t,
             "txt_a": txt_a, "txt_b": tl, "yext": yext}
        )
    return in_maps


def kernel(x, train_X, Y, W1, W2, h, *, _trace=False):
    x = np.asarray(x, dtype=np.float32)
    train_X = np.asarray(train_X, dtype=np.float32)
    Y = np.asarray(Y, dtype=np.float32)
    W1 = np.asarray(W1, dtype=np.float32)
    W2 = np.asarray(W2, dtype=np.float32)
    h = np.asarray(h, dtype=np.float32)

    c = float(-0.5 / (h[0] ** 2))
    key = (c,)
    if key not in _cache:
        _cache[key] = _build(c)
    nc = _cache[key]

    in_maps = _prep_inputs(x, train_X, Y, W1, W2)
    res = run_bass_kernel_spmd(
        nc, in_maps, core_ids=list(range(N_CORES)), trace=_trace
    )
    parts = np.stack([r["out"] for r in res.results])  # [8, 20, 512]
    tot = parts.sum(axis=0, dtype=np.float64)
    final = np.ascontiguousarray(
        (tot[:DOUT] / tot[DOUT:]).T.astype(np.float32)
    )
    if _trace:
        return final, res
    return final


# revision 30
# speedup vs baseline: 1.0252x; 1.0252x over previous
"""Trainium2 Bass kernel for Nadaraya-Watson kernel regression over MLP embeddings.

Computes: out[b,d] = sum_n K[n,b,d]*Y[n,d] / sum_n K[n,b,d]
where K = exp(-0.5*((z[n,d]-zw[b,d])/h)^2), z/zw are 2-layer MLP embeddings of
train/query points (ReLU MLP 64->128->10).

Strategy (8 NeuronCores, data-parallel over the train axis N, ~100us/core):
 - shard train_X/Y over 8 cores (1250 rows each, padded to 1280 = 10 tiles
   of 128); pad rows are masked via the ones-column of the Y-side weights.
 - layer-1 embedding operands are hi/lo bf16-split on the host (exact fp32
   products in the fp32 PSUM accumulator at full bf16 PE rate); layer 2 runs
   fp32. Embeddings are bit-accurate to ~1e-5.
 - u^2 = z^2 - 2 z zw + zw^2 is built as a K=7 bf16 matmul per (d, n-tile)
   from hi/lo bf16 splits of z^2, -2z, zw, zw^2 (u^2 accurate to ~1e-4),
   streamed at full PE rate; exp on ScalarE with scale=-0.5/h^2 folded in,
   over 3-PSUM-bank [128,1536] chunks (ScalarE is the ~53us bottleneck).
 - PE array packing: u^2 matmuls 2-way row-tiled (partition offsets 0/64,
   operands replicated there via one contiguous DRAM-bounce DMA per group);
   numerator+denominator reduce matmuls (M=2, contracting n on partitions)
   4-way column-tiled (t%4) into one PSUM accumulator bank per d.
 - the main loop is software-pipelined over flat uniform 3-tile chunks:
   reduce matmuls of chunk k are emitted after the u^2 matmuls + exp of
   chunk k+1 so the in-order PE queue never stalls on ScalarE.
 - each core outputs [20,512] partials (10 numerator rows, 10 denominator
   rows); the host sums the 8 partials, divides, transposes -- the
   data-parallel all-reduce + unshard step of the hint.
"""

import ml_dtypes
import numpy as np

import concourse.bass as bass
import concourse.tile as tile
from concourse import bacc, mybir
from concourse.bass_utils import run_bass_kernel_spmd

F32 = mybir.dt.float32
BF16 = mybir.dt.bfloat16
AF = mybir.ActivationFunctionType

N_CORES = 8
B = 512          # batch (queries)
DOUT = 10
DIN = 64
DHID = 128
N_TRAIN = 10000
NSH = 1280       # padded per-core shard rows
NT = NSH // 128  # 10 n-tiles per core
SHARD = N_TRAIN // N_CORES  # 1250 valid rows per core

_cache = {}


def _build(c: float):
    """Build + compile the per-core Bass kernel. c = -0.5/h^2."""
    nc = bacc.Bacc(
        "TRN2",
        target_bir_lowering=False,
        debug=False,
        enable_asserts=False,
        num_devices=N_CORES,
    )

    # layer-1 operands arrive hi/lo bf16-split from the host:
    #   xt_a/txt_a = [Xhi; Xhi] (dup at partition 64), xt_b/txt_b = Xlo
    #   w1s = [W1hiT; W1loT]
    xta_d = nc.dram_tensor("xt_a", [2 * DIN, B], BF16, kind="ExternalInput")
    xtb_d = nc.dram_tensor("xt_b", [DIN, B], BF16, kind="ExternalInput")
    txta_d = nc.dram_tensor("txt_a", [2 * DIN, NSH], BF16, kind="ExternalInput")
    txtb_d = nc.dram_tensor("txt_b", [DIN, NSH], BF16, kind="ExternalInput")
    w1s_d = nc.dram_tensor("w1s", [2 * DIN, DHID], BF16, kind="ExternalInput")
    w2t_d = nc.dram_tensor("w2t", [DHID, DOUT], F32, kind="ExternalInput")   # W2^T
    yext_d = nc.dram_tensor("yext", [NSH, 2 * DOUT], F32, kind="ExternalInput")
    out_d = nc.dram_tensor("out", [2 * DOUT, B], F32, kind="ExternalOutput")

    with tile.TileContext(nc) as tc:
        with (
            tc.tile_pool(name="const", bufs=1) as const,
            tc.tile_pool(name="work", bufs=1) as work,
            tc.tile_pool(name="sums", bufs=4) as sums,
            tc.tile_pool(name="psB", bufs=2, space=bass.MemorySpace.PSUM) as psB,
            tc.tile_pool(name="accp", bufs=2, space=bass.MemorySpace.PSUM) as accp,
            tc.tile_pool(name="ksbp", bufs=3) as ksbp,
            tc.tile_pool(name="dram", bufs=1, space="DRAM") as dram,
        ):
            # ---- load inputs (spread across DMA queues) ----
            # W1^T / trainX^T replicated at partitions 0 and 64 for 2-way
            # row-tiled K=64 embedding matmuls.
            sb_txta = const.tile([2 * DIN, NSH], BF16)
            nc.sync.dma_start(sb_txta[:], txta_d.ap())
            sb_txtb = const.tile([DIN, NSH], BF16)
            nc.sync.dma_start(sb_txtb[:], txtb_d.ap())
            sb_w1s = const.tile([2 * DIN, DHID], BF16)
            nc.scalar.dma_start(sb_w1s[:], w1s_d.ap())
            sb_w2t = const.tile([DHID, DOUT], F32)
            nc.scalar.dma_start(sb_w2t[:], w2t_d.ap())
            sb_xta = const.tile([2 * DIN, B], BF16)
            nc.scalar.dma_start(sb_xta[:], xta_d.ap())
            sb_xtb = const.tile([DIN, B], BF16)
            nc.scalar.dma_start(sb_xtb[:], xtb_d.ap())
            # yext [1280, 20] -> [128, 10, 20] (n-within-tile on partitions)
            sb_y32 = const.tile([128, NT, 2 * DOUT], F32)
            nc.gpsimd.dma_start(
                sb_y32[:], yext_d.ap().rearrange("(t p) c -> p t c", p=128)
            )
            sb_yb = const.tile([128, NT, 2 * DOUT], BF16)
            nc.vector.tensor_copy(sb_yb[:], sb_y32[:])

            # ---- embeddings ----
            # PE order: Ht -> Hq -> Zt -> Zq so the (longer) phi chain starts
            # as early as possible and the psi chain overlaps it.
            ps_ht = psB.tile([128, 1536], F32, tag="pu")
            for a, b_ in ((0, 512), (512, 1024), (1024, 1280)):
                nc.tensor.matmul(
                    ps_ht[:, a:b_], sb_w1s[:], sb_txta[:, a:b_],
                    start=True, stop=False,
                )
                nc.tensor.matmul(
                    ps_ht[:, a:b_], sb_w1s[0:DIN, :], sb_txtb[:, a:b_],
                    start=False, stop=True,
                )
            sb_ht = work.tile([DHID, NSH], F32)
            for a, b_ in ((0, 512), (512, 1024), (1024, 1280)):
                nc.scalar.activation(sb_ht[:, a:b_], ps_ht[:, a:b_], AF.Relu)

            # Zt matmuls (M=10) use col tiles 0/32/64 for 3-way concurrency.
            # sb_zt is stored group-major: n' = (g, t5, p) with tile t = 2*t5+g,
            # so the phi scratch -> replica DMAs are fully contiguous per group.
            ps_zt = psB.tile([128, 1536], F32, tag="pu")
            for i, (a, b_) in enumerate(((0, 512), (512, 1024), (1024, 1280))):
                nc.tensor.matmul(
                    ps_zt[32 * i:32 * i + DOUT, a:b_], sb_w2t[:], sb_ht[:, a:b_],
                    start=True, stop=True, tile_position=(0, 32 * i),
                )
            ps_hq = psB.tile([128, 1536], F32, tag="pu")
            nc.tensor.matmul(
                ps_hq[:, :B], sb_w1s[:], sb_xta[:], start=True, stop=False
            )
            nc.tensor.matmul(
                ps_hq[:, :B], sb_w1s[0:DIN, :], sb_xtb[:], start=False, stop=True
            )
            sb_hq = work.tile([DHID, B], F32)
            nc.vector.tensor_relu(sb_hq[:], ps_hq[:, :B])
            ps_zq = psB.tile([128, 1536], F32, tag="pu")
            nc.tensor.matmul(
                ps_zq[0:DOUT, :B], sb_w2t[:], sb_hq[:], start=True, stop=True
            )

            sb_zt2 = work.tile([DOUT, 2, 5, 128], F32)
            nc.vector.tensor_copy(
                sb_zt2[:, :, 0:2, :].rearrange("d g t p -> d t g p"),
                ps_zt[0:DOUT, 0:512].rearrange("d (t g p) -> d t g p", g=2, p=128),
            )
            nc.scalar.activation(
                sb_zt2[:, :, 2:4, :].rearrange("d g t p -> d t g p"),
                ps_zt[32:32 + DOUT, 512:1024].rearrange(
                    "d (t g p) -> d t g p", g=2, p=128
                ),
                AF.Copy,
            )
            nc.scalar.activation(
                sb_zt2[:, :, 4, :],
                ps_zt[64:64 + DOUT, 1024:1280].rearrange(
                    "d (g p) -> d g p", p=128
                ),
                AF.Copy,
            )
            sb_zt = sb_zt2[:].rearrange("d g t p -> d (g t p)")
            sb_zq = work.tile([DOUT, B], F32)
            nc.vector.tensor_copy(sb_zq[:], ps_zq[0:DOUT, :B])

            # ---- phi pieces (train side), [10, 1280] ops ----
            # u^2[n,b] = sh*1 + sl*1 + 1*qh + 1*ql + m2zh*wh + m2zh*wl + m2zl*wh
            # staging rows phi: 0 sh, 1 sl, 2 ones, 3 ones, 4 m2zh, 5 m2zh, 6 m2zl
            stg_phi = work.tile([DOUT, 7, 1280], BF16)
            # All ops split into n'-halves (= row-tile groups, since sb_zt is
            # group-major): the g0 half finishes first so its scratch bounce +
            # replica DMAs overlap the g1 half of the chain.
            t_sq = work.tile([DOUT, NSH], F32)
            for h0, h1 in ((0, 640), (640, 1280)):
                # m2zh = bf16(-2 z) via ACT scale; m2zl fused on DVE
                nc.scalar.activation(
                    stg_phi[:, 4, h0:h1], sb_zt[:, h0:h1], AF.Copy, scale=-2.0
                )
                nc.vector.scalar_tensor_tensor(
                    stg_phi[:, 6, h0:h1], sb_zt[:, h0:h1], -2.0,
                    stg_phi[:, 4, h0:h1],
                    op0=mybir.AluOpType.mult, op1=mybir.AluOpType.subtract,
                )
                # sh = bf16(z^2) on ACT in parallel with exact fp32 square on DVE
                nc.vector.tensor_mul(t_sq[:, h0:h1], sb_zt[:, h0:h1], sb_zt[:, h0:h1])
                nc.scalar.activation(stg_phi[:, 0, h0:h1], sb_zt[:, h0:h1], AF.Square)
                nc.vector.tensor_sub(
                    stg_phi[:, 1, h0:h1], t_sq[:, h0:h1], stg_phi[:, 0, h0:h1]
                )
            nc.gpsimd.memset(stg_phi[:, 2:4, :], 1.0)  # ones rows

            # ---- psi pieces (query side) ----
            # psi staging rows: 0 ones, 1 ones, 2 qh, 3 ql, 4 wh, 5 wl, 6 wh
            stg_psi = work.tile([DOUT, 7, B], BF16)
            nc.scalar.activation(stg_psi[:, 4, :], sb_zq[:], AF.Copy)
            nc.vector.tensor_sub(stg_psi[:, 5, :], sb_zq[:], stg_psi[:, 4, :])
            t_qw = work.tile([DOUT, B], F32)
            nc.vector.tensor_mul(t_qw[:], sb_zq[:], sb_zq[:])
            nc.scalar.activation(stg_psi[:, 2, :], sb_zq[:], AF.Square)
            nc.vector.tensor_sub(stg_psi[:, 3, :], t_qw[:], stg_psi[:, 2, :])
            nc.gpsimd.memset(stg_psi[:, 0:2, :], 1.0)

            # ---- reorder [d, r, n] -> [r, d, n] via DRAM bounce; replicate at
            # partition offsets 0/32/64/96 for row-tiled u^2 matmuls ----
            # Row-tile group of tile t is g2 = t%2 at partition offsets 0/64.
            # sb_zt's group-major n-order makes scratch and replicas fully
            # contiguous: one bounce + one replica DMA per group. Duplicated
            # rows (phi 5<-4, psi 6<-4) are materialized by the bounce DMAs.
            phic = const.tile([71, DOUT, 5, 128], BF16)
            psic = const.tile([71, DOUT, B], BF16)
            scr_phi = dram.tile([DOUT, 7, 2, 5, 128], BF16)
            scr_psi = dram.tile([7, DOUT, B], BF16)
            scr_v = scr_phi[:].rearrange("d r g t5 p -> d r (g t5 p)")
            for h0, h1 in ((0, 640), (640, 1280)):
                nc.sync.dma_start(scr_v[:, 0:5, h0:h1], stg_phi[:, 0:5, h0:h1])
                nc.sync.dma_start(scr_v[:, 5:6, h0:h1], stg_phi[:, 4:5, h0:h1])
                nc.sync.dma_start(scr_v[:, 6:7, h0:h1], stg_phi[:, 6:7, h0:h1])
            psi_v = scr_psi[:].rearrange("r d b -> d r b")
            nc.scalar.dma_start(psi_v[:, 6:7, :], stg_psi[:, 4:5, :])
            nc.scalar.dma_start(psi_v[:, 0:6, :], stg_psi[:, 0:6, :])
            nc.scalar.dma_start(psic[0:7], scr_psi[:])
            nc.gpsimd.dma_start(psic[64:71], scr_psi[:])
            for g in range(2):
                eng = (nc.sync, nc.scalar)[g]
                eng.dma_start(
                    phic[64 * g:64 * g + 7],
                    scr_phi[:, :, g].rearrange("d r t5 p -> r d t5 p"),
                )

            # ---- software-pipelined main loop ----
            # flat uniform chunks of 3 tiles across the whole (d, t) list --
            # no per-d ragged chunk, so the exp pipeline never hiccups at a
            # d boundary.
            flat = [(d, t) for d in range(DOUT) for t in range(NT)]
            steps = [tuple(flat[i:i + 3]) for i in range(0, len(flat), 3)]
            res2 = work.tile([2, DOUT, B], F32)
            out_v = out_d.ap().rearrange("(two dd) b -> two dd b", two=2)
            kts = {}
            accs = {}
            for k in range(len(steps) + 1):
                if k < len(steps):
                    tt = steps[k]
                    pu = psB.tile([128, 1536], F32, tag="pu")
                    for j, (d, t) in enumerate(tt):
                        g = t % 2
                        nc.tensor.matmul(
                            pu[:, j * B:(j + 1) * B],
                            phic[64 * g:64 * g + 7, d, t // 2, :],
                            psic[64 * g:64 * g + 7, d, :],
                            start=True,
                            stop=True,
                            tile_position=(64 * g, 0),
                        )
                    kt = ksbp.tile([128, 1536], BF16, tag="kt")
                    nc.scalar.activation(
                        kt[:, : len(tt) * B], pu[:, : len(tt) * B], AF.Exp, scale=c
                    )
                    kts[k] = (kt, tt)
                if k > 0:
                    kt, tt = kts.pop(k - 1)
                    for j, (d, t) in enumerate(tt):
                        # column group gc = t%4: group gc accumulates tiles
                        # {gc, gc+4, gc+8}; distinct groups per chunk overlap.
                        if d not in accs:
                            acc_t = accp.tile([98, B], F32, tag="acc")
                            accs[d] = acc_t
                        # final d accumulates in a single group so its tail
                        # drain is one ScalarE copy instead of a DVE add chain
                        gc = 0 if d == DOUT - 1 else t % 4
                        nc.tensor.matmul(
                            accs[d][32 * gc:32 * gc + 2, :],
                            sb_yb[:, t, 2 * d:2 * d + 2],
                            kt[:, j * B:(j + 1) * B],
                            start=(t == 0 if d == DOUT - 1 else t < 4),
                            stop=(t == NT - 1 if d == DOUT - 1 else t >= NT - 4),
                            tile_position=(0, 32 * gc),
                        )
                    d9 = [d for (d, t) in tt if t == NT - 1]
                    if d9:
                        # d complete: sum the 4 column-tile partials
                        # (DVE reads at most one PSUM operand per op)
                        d = d9[0]
                        acc4 = accs.pop(d)
                        if d == DOUT - 1:
                            nc.scalar.activation(
                                res2[:, d, :], acc4[0:2, :], AF.Copy
                            )
                        else:
                            s0 = sums.tile([2, B], F32, tag="s0")
                            nc.vector.tensor_copy(s0[:], acc4[0:2, :])
                            s1 = sums.tile([2, B], F32, tag="s1")
                            nc.vector.tensor_add(s1[:], s0[:], acc4[32:34, :])
                            s2 = sums.tile([2, B], F32, tag="s2")
                            nc.vector.tensor_add(s2[:], s1[:], acc4[64:66, :])
                            nc.vector.tensor_add(
                                res2[:, d, :], s2[:], acc4[96:98, :]
                            )
                        nc.sync.dma_start(out_v[:, d, :], res2[:, d, :])

    nc.compile()
    return nc


def _split_hi_lo(a32):
    hi = a32.astype(ml_dtypes.bfloat16)
    lo = (a32 - hi.astype(np.float32)).astype(ml_dtypes.bfloat16)
    return hi, lo


def _prep_inputs(x, train_X, Y, W1, W2):
    xh, xl = _split_hi_lo(np.ascontiguousarray(x.T, dtype=np.float32))
    xt_a = np.concatenate([xh, xh], axis=0)
    w1h, w1l = _split_hi_lo(np.ascontiguousarray(W1.T, dtype=np.float32))
    w1s = np.concatenate([w1h, w1l], axis=0)
    w2t = np.ascontiguousarray(W2.T, dtype=np.float32)
    in_maps = []
    for i in range(N_CORES):
        sl = slice(i * SHARD, (i + 1) * SHARD)
        txt = np.zeros((DIN, NSH), dtype=np.float32)
        txt[:, :SHARD] = train_X[sl].T
        th, tl = _split_hi_lo(txt)
        txt_a = np.concatenate([th, th], axis=0)
        yext = np.zeros((NSH, 2 * DOUT), dtype=np.float32)
        yext[:SHARD, 0::2] = Y[sl]
        yext[:SHARD, 1::2] = 1.0
        in_maps.append(
            {"xt_a": xt_a, "xt_b": xl, "w1s": w1s, "w2t": w2t,
             "txt_a": txt_a, "txt_b": tl, "yext": yext}
        )
    return in_maps


def kernel(x, train_X, Y, W1, W2, h, *, _trace=False):
    x = np.asarray(x, dtype=np.float32)
    train_X = np.asarray(train_X, dtype=np.float32)
    Y = np.asarray(Y, dtype=np.float32)
    W1 = np.asarray(W1, dtype=np.float32)
    W2 = np.asarray(W2, dtype=np.float32)
    h = np.asarray(h, dtype=np.float32)

    c = float(-0.5 / (h[0] ** 2))
    key = (c,)
    if key not in _cache:
        _cache[key] = _build(c)
    nc = _cache[key]

    in_maps = _prep_inputs(x, train_X, Y, W1, W2)
    res = run_bass_kernel_spmd(
        nc, in_maps, core_ids=list(range(N_CORES)), trace=_trace
    )
    parts = np.stack([r["out"] for r in res.results])  # [8, 20, 512]
    tot = parts.sum(axis=0, dtype=np.float64)
    final = np.ascontiguousarray(
        (tot[:DOUT] / tot[DOUT:]).T.astype(np.float32)
    )
    if _trace:
        return final, res
    return final


# revision 31
# speedup vs baseline: 1.0317x; 1.0064x over previous
"""Trainium2 Bass kernel for Nadaraya-Watson kernel regression over MLP embeddings.

Computes: out[b,d] = sum_n K[n,b,d]*Y[n,d] / sum_n K[n,b,d]
where K = exp(-0.5*((z[n,d]-zw[b,d])/h)^2), z/zw are 2-layer MLP embeddings of
train/query points (ReLU MLP 64->128->10).

Strategy (8 NeuronCores, data-parallel over the train axis N, ~100us/core):
 - shard train_X/Y over 8 cores (1250 rows each, padded to 1280 = 10 tiles
   of 128); pad rows are masked via the ones-column of the Y-side weights.
 - layer-1 embedding operands are hi/lo bf16-split on the host (exact fp32
   products in the fp32 PSUM accumulator at full bf16 PE rate); layer 2 runs
   fp32. Embeddings are bit-accurate to ~1e-5.
 - u^2 = z^2 - 2 z zw + zw^2 is built as a K=7 bf16 matmul per (d, n-tile)
   from hi/lo bf16 splits of z^2, -2z, zw, zw^2 (u^2 accurate to ~1e-4),
   streamed at full PE rate; exp on ScalarE with scale=-0.5/h^2 folded in,
   over 3-PSUM-bank [128,1536] chunks (ScalarE is the ~53us bottleneck).
 - PE array packing: u^2 matmuls 2-way row-tiled (partition offsets 0/64,
   operands replicated there via one contiguous DRAM-bounce DMA per group);
   numerator+denominator reduce matmuls (M=2, contracting n on partitions)
   4-way column-tiled (t%4) into one PSUM accumulator bank per d.
 - the main loop is software-pipelined over flat uniform 3-tile chunks:
   reduce matmuls of chunk k are emitted after the u^2 matmuls + exp of
   chunk k+1 so the in-order PE queue never stalls on ScalarE.
 - each core outputs [20,512] partials (10 numerator rows, 10 denominator
   rows); the host sums the 8 partials, divides, transposes -- the
   data-parallel all-reduce + unshard step of the hint.
"""

import ml_dtypes
import numpy as np

import concourse.bass as bass
import concourse.tile as tile
from concourse import bacc, mybir
from concourse.bass_utils import run_bass_kernel_spmd

F32 = mybir.dt.float32
BF16 = mybir.dt.bfloat16
AF = mybir.ActivationFunctionType

N_CORES = 8
B = 512          # batch (queries)
DOUT = 10
DIN = 64
DHID = 128
N_TRAIN = 10000
NSH = 1280       # padded per-core shard rows
NT = NSH // 128  # 10 n-tiles per core
SHARD = N_TRAIN // N_CORES  # 1250 valid rows per core

_cache = {}


def _build(c: float):
    """Build + compile the per-core Bass kernel. c = -0.5/h^2."""
    nc = bacc.Bacc(
        "TRN2",
        target_bir_lowering=False,
        debug=False,
        enable_asserts=False,
        num_devices=N_CORES,
    )

    # layer-1 operands arrive hi/lo bf16-split from the host:
    #   xt_a/txt_a = [Xhi; Xhi] (dup at partition 64), xt_b/txt_b = Xlo
    #   w1s = [W1hiT; W1loT]
    xta_d = nc.dram_tensor("xt_a", [2 * DIN, B], BF16, kind="ExternalInput")
    xtb_d = nc.dram_tensor("xt_b", [DIN, B], BF16, kind="ExternalInput")
    txta_d = nc.dram_tensor("txt_a", [2 * DIN, NSH], BF16, kind="ExternalInput")
    txtb_d = nc.dram_tensor("txt_b", [DIN, NSH], BF16, kind="ExternalInput")
    w1s_d = nc.dram_tensor("w1s", [2 * DIN, DHID], BF16, kind="ExternalInput")
    w2t_d = nc.dram_tensor("w2t", [DHID, DOUT], F32, kind="ExternalInput")   # W2^T
    yext_d = nc.dram_tensor("yext", [NSH, 2 * DOUT], F32, kind="ExternalInput")
    out_d = nc.dram_tensor("out", [2 * DOUT, B], F32, kind="ExternalOutput")

    with tile.TileContext(nc) as tc:
        with (
            tc.tile_pool(name="const", bufs=1) as const,
            tc.tile_pool(name="work", bufs=1) as work,
            tc.tile_pool(name="sums", bufs=4) as sums,
            tc.tile_pool(name="psB", bufs=2, space=bass.MemorySpace.PSUM) as psB,
            tc.tile_pool(name="accp", bufs=2, space=bass.MemorySpace.PSUM) as accp,
            tc.tile_pool(name="ksbp", bufs=3) as ksbp,
            tc.tile_pool(name="dram", bufs=1, space="DRAM") as dram,
        ):
            # ---- load inputs (spread across DMA queues) ----
            # W1^T / trainX^T replicated at partitions 0 and 64 for 2-way
            # row-tiled K=64 embedding matmuls.
            sb_txta = const.tile([2 * DIN, NSH], BF16)
            nc.sync.dma_start(sb_txta[:], txta_d.ap())
            sb_txtb = const.tile([DIN, NSH], BF16)
            nc.sync.dma_start(sb_txtb[:], txtb_d.ap())
            sb_w1s = const.tile([2 * DIN, DHID], BF16)
            nc.scalar.dma_start(sb_w1s[:], w1s_d.ap())
            sb_w2t = const.tile([DHID, DOUT], F32)
            nc.scalar.dma_start(sb_w2t[:], w2t_d.ap())
            sb_xta = const.tile([2 * DIN, B], BF16)
            nc.scalar.dma_start(sb_xta[:], xta_d.ap())
            sb_xtb = const.tile([DIN, B], BF16)
            nc.scalar.dma_start(sb_xtb[:], xtb_d.ap())
            # yext [1280, 20] -> [128, 10, 20] (n-within-tile on partitions)
            sb_y32 = const.tile([128, NT, 2 * DOUT], F32)
            nc.gpsimd.dma_start(
                sb_y32[:], yext_d.ap().rearrange("(t p) c -> p t c", p=128)
            )
            sb_yb = const.tile([128, NT, 2 * DOUT], BF16)
            nc.vector.tensor_copy(sb_yb[:], sb_y32[:])

            # ---- embeddings ----
            # PE order: Ht -> Hq -> Zt -> Zq so the (longer) phi chain starts
            # as early as possible and the psi chain overlaps it.
            ps_ht = psB.tile([128, 1536], F32, tag="pu")
            for a, b_ in ((0, 512), (512, 1024), (1024, 1280)):
                nc.tensor.matmul(
                    ps_ht[:, a:b_], sb_w1s[:], sb_txta[:, a:b_],
                    start=True, stop=False,
                )
                nc.tensor.matmul(
                    ps_ht[:, a:b_], sb_w1s[0:DIN, :], sb_txtb[:, a:b_],
                    start=False, stop=True,
                )
            sb_ht = work.tile([DHID, NSH], F32)
            for a, b_ in ((0, 512), (512, 1024), (1024, 1280)):
                nc.scalar.activation(sb_ht[:, a:b_], ps_ht[:, a:b_], AF.Relu)

            # Zt matmuls (M=10) use col tiles 0/32/64 for 3-way concurrency.
            # sb_zt is stored group-major: n' = (g, t5, p) with tile t = 2*t5+g,
            # so the phi scratch -> replica DMAs are fully contiguous per group.
            ps_zt = psB.tile([128, 1536], F32, tag="pu")
            for i, (a, b_) in enumerate(((0, 512), (512, 1024), (1024, 1280))):
                nc.tensor.matmul(
                    ps_zt[32 * i:32 * i + DOUT, a:b_], sb_w2t[:], sb_ht[:, a:b_],
                    start=True, stop=True, tile_position=(0, 32 * i),
                )
            ps_hq = psB.tile([128, 1536], F32, tag="pu")
            nc.tensor.matmul(
                ps_hq[:, :B], sb_w1s[:], sb_xta[:], start=True, stop=False
            )
            nc.tensor.matmul(
                ps_hq[:, :B], sb_w1s[0:DIN, :], sb_xtb[:], start=False, stop=True
            )
            sb_hq = work.tile([DHID, B], F32)
            nc.vector.tensor_relu(sb_hq[:], ps_hq[:, :B])
            ps_zq = psB.tile([128, 1536], F32, tag="pu")
            nc.tensor.matmul(
                ps_zq[0:DOUT, :B], sb_w2t[:], sb_hq[:], start=True, stop=True
            )

            # sb_zt is stored t%4-group-major with tiles-per-group padded to
            # 3 (slots (g,2) for g>=2 are dead): n' = (g4, t4, p), t = 4*t4+g4.
            sb_zt2 = work.tile([DOUT, 4, 3, 128], F32)
            nc.vector.tensor_copy(sb_zt2[:, :, 0, :], ps_zt[0:DOUT, 0:512])
            nc.scalar.activation(
                sb_zt2[:, :, 1, :], ps_zt[32:32 + DOUT, 512:1024], AF.Copy
            )
            nc.scalar.activation(
                sb_zt2[:, 0:2, 2, :], ps_zt[64:64 + DOUT, 1024:1280], AF.Copy
            )
            sb_zt = sb_zt2[:].rearrange("d g t p -> d (g t p)")
            sb_zq = work.tile([DOUT, B], F32)
            nc.vector.tensor_copy(sb_zq[:], ps_zq[0:DOUT, :B])

            # ---- phi pieces (train side), [10, 1280] ops ----
            # u^2[n,b] = sh*1 + sl*1 + 1*qh + 1*ql + m2zh*wh + m2zh*wl + m2zl*wh
            # staging rows phi: 0 sh, 1 sl, 2 ones, 3 ones, 4 m2zh, 5 m2zh, 6 m2zl
            stg_phi = work.tile([DOUT, 7, 1536], BF16)
            # All ops split into n'-halves (= row-tile group pairs): the first
            # half finishes earlier so its scratch bounce + replica DMAs
            # overlap the second half of the chain. Dead pad slots carry
            # garbage that no matmul ever reads.
            t_sq = work.tile([DOUT, 1536], F32)
            for h0, h1 in ((0, 768), (768, 1536)):
                # m2zh = bf16(-2 z) via ACT scale; m2zl fused on DVE
                nc.scalar.activation(
                    stg_phi[:, 4, h0:h1], sb_zt[:, h0:h1], AF.Copy, scale=-2.0
                )
                nc.vector.scalar_tensor_tensor(
                    stg_phi[:, 6, h0:h1], sb_zt[:, h0:h1], -2.0,
                    stg_phi[:, 4, h0:h1],
                    op0=mybir.AluOpType.mult, op1=mybir.AluOpType.subtract,
                )
                # sh = bf16(z^2) on ACT in parallel with exact fp32 square on DVE
                nc.vector.tensor_mul(t_sq[:, h0:h1], sb_zt[:, h0:h1], sb_zt[:, h0:h1])
                nc.scalar.activation(stg_phi[:, 0, h0:h1], sb_zt[:, h0:h1], AF.Square)
                nc.vector.tensor_sub(
                    stg_phi[:, 1, h0:h1], t_sq[:, h0:h1], stg_phi[:, 0, h0:h1]
                )
            nc.gpsimd.memset(stg_phi[:, 2:4, :], 1.0)  # ones rows

            # ---- psi pieces (query side) ----
            # psi staging rows: 0 ones, 1 ones, 2 qh, 3 ql, 4 wh, 5 wl, 6 wh
            stg_psi = work.tile([DOUT, 7, B], BF16)
            nc.scalar.activation(stg_psi[:, 4, :], sb_zq[:], AF.Copy)
            nc.vector.tensor_sub(stg_psi[:, 5, :], sb_zq[:], stg_psi[:, 4, :])
            t_qw = work.tile([DOUT, B], F32)
            nc.vector.tensor_mul(t_qw[:], sb_zq[:], sb_zq[:])
            nc.scalar.activation(stg_psi[:, 2, :], sb_zq[:], AF.Square)
            nc.vector.tensor_sub(stg_psi[:, 3, :], t_qw[:], stg_psi[:, 2, :])
            nc.gpsimd.memset(stg_psi[:, 0:2, :], 1.0)

            # ---- reorder [d, r, n] -> [r, d, n] via DRAM bounce; replicate at
            # partition offsets 0/32/64/96 for row-tiled u^2 matmuls ----
            # Row-tile group of tile t is g2 = t%2 at partition offsets 0/64.
            # sb_zt's group-major n-order makes scratch and replicas fully
            # contiguous: one bounce + one replica DMA per group. Duplicated
            # rows (phi 5<-4, psi 6<-4) are materialized by the bounce DMAs.
            phic = const.tile([103, DOUT, 3, 128], BF16)
            psic = const.tile([103, DOUT, B], BF16)
            scr_phi = dram.tile([DOUT, 7, 4, 3, 128], BF16)
            scr_psi = dram.tile([7, DOUT, B], BF16)
            scr_v = scr_phi[:].rearrange("d r g t4 p -> d r (g t4 p)")
            for h0, h1 in ((0, 768), (768, 1536)):
                nc.sync.dma_start(scr_v[:, 0:5, h0:h1], stg_phi[:, 0:5, h0:h1])
                nc.sync.dma_start(scr_v[:, 5:6, h0:h1], stg_phi[:, 4:5, h0:h1])
                nc.sync.dma_start(scr_v[:, 6:7, h0:h1], stg_phi[:, 6:7, h0:h1])
            psi_v = scr_psi[:].rearrange("r d b -> d r b")
            nc.scalar.dma_start(psi_v[:, 6:7, :], stg_psi[:, 4:5, :])
            nc.scalar.dma_start(psi_v[:, 0:6, :], stg_psi[:, 0:6, :])
            for g in range(4):
                eng = (nc.scalar, nc.gpsimd, nc.scalar, nc.gpsimd)[g]
                eng.dma_start(psic[32 * g:32 * g + 7], scr_psi[:])
                eng2 = (nc.sync, nc.scalar)[g % 2]
                eng2.dma_start(
                    phic[32 * g:32 * g + 7],
                    scr_phi[:, :, g].rearrange("d r t4 p -> r d t4 p"),
                )

            # ---- software-pipelined main loop ----
            # flat uniform chunks of 3 tiles across the whole (d, t) list --
            # no per-d ragged chunk, so the exp pipeline never hiccups at a
            # d boundary.
            flat = [(d, t) for d in range(DOUT) for t in range(NT)]
            steps = [tuple(flat[i:i + 3]) for i in range(0, len(flat), 3)]
            res2 = work.tile([2, DOUT, B], F32)
            out_v = out_d.ap().rearrange("(two dd) b -> two dd b", two=2)
            kts = {}
            accs = {}
            for k in range(len(steps) + 1):
                if k < len(steps):
                    tt = steps[k]
                    pu = psB.tile([128, 1536], F32, tag="pu")
                    for j, (d, t) in enumerate(tt):
                        g = t % 4
                        nc.tensor.matmul(
                            pu[:, j * B:(j + 1) * B],
                            phic[32 * g:32 * g + 7, d, t // 4, :],
                            psic[32 * g:32 * g + 7, d, :],
                            start=True,
                            stop=True,
                            tile_position=(32 * g, 0),
                        )
                    kt = ksbp.tile([128, 1536], BF16, tag="kt")
                    nc.scalar.activation(
                        kt[:, : len(tt) * B], pu[:, : len(tt) * B], AF.Exp, scale=c
                    )
                    kts[k] = (kt, tt)
                if k > 0:
                    kt, tt = kts.pop(k - 1)
                    for j, (d, t) in enumerate(tt):
                        # column group gc = t%4: group gc accumulates tiles
                        # {gc, gc+4, gc+8}; distinct groups per chunk overlap.
                        if d not in accs:
                            acc_t = accp.tile([98, B], F32, tag="acc")
                            accs[d] = acc_t
                        # final d accumulates in a single group so its tail
                        # drain is one ScalarE copy instead of a DVE add chain
                        gc = 0 if d == DOUT - 1 else t % 4
                        nc.tensor.matmul(
                            accs[d][32 * gc:32 * gc + 2, :],
                            sb_yb[:, t, 2 * d:2 * d + 2],
                            kt[:, j * B:(j + 1) * B],
                            start=(t == 0 if d == DOUT - 1 else t < 4),
                            stop=(t == NT - 1 if d == DOUT - 1 else t >= NT - 4),
                            tile_position=(0, 32 * gc),
                        )
                    d9 = [d for (d, t) in tt if t == NT - 1]
                    if d9:
                        # d complete: sum the 4 column-tile partials
                        # (DVE reads at most one PSUM operand per op)
                        d = d9[0]
                        acc4 = accs.pop(d)
                        if d == DOUT - 1:
                            nc.scalar.activation(
                                res2[:, d, :], acc4[0:2, :], AF.Copy
                            )
                        else:
                            s0 = sums.tile([2, B], F32, tag="s0")
                            nc.vector.tensor_copy(s0[:], acc4[0:2, :])
                            s1 = sums.tile([2, B], F32, tag="s1")
                            nc.vector.tensor_add(s1[:], s0[:], acc4[32:34, :])
                            s2 = sums.tile([2, B], F32, tag="s2")
                            nc.vector.tensor_add(s2[:], s1[:], acc4[64:66, :])
                            nc.vector.tensor_add(
                                res2[:, d, :], s2[:], acc4[96:98, :]
                            )
                        nc.sync.dma_start(out_v[:, d, :], res2[:, d, :])

    nc.compile()
    return nc


def _split_hi_lo(a32):
    hi = a32.astype(ml_dtypes.bfloat16)
    lo = (a32 - hi.astype(np.float32)).astype(ml_dtypes.bfloat16)
    return hi, lo


def _prep_inputs(x, train_X, Y, W1, W2):
    xh, xl = _split_hi_lo(np.ascontiguousarray(x.T, dtype=np.float32))
    xt_a = np.concatenate([xh, xh], axis=0)
    w1h, w1l = _split_hi_lo(np.ascontiguousarray(W1.T, dtype=np.float32))
    w1s = np.concatenate([w1h, w1l], axis=0)
    w2t = np.ascontiguousarray(W2.T, dtype=np.float32)
    in_maps = []
    for i in range(N_CORES):
        sl = slice(i * SHARD, (i + 1) * SHARD)
        txt = np.zeros((DIN, NSH), dtype=np.float32)
        txt[:, :SHARD] = train_X[sl].T
        th, tl = _split_hi_lo(txt)
        txt_a = np.concatenate([th, th], axis=0)
        yext = np.zeros((NSH, 2 * DOUT), dtype=np.float32)
        yext[:SHARD, 0::2] = Y[sl]
        yext[:SHARD, 1::2] = 1.0
        in_maps.append(
            {"xt_a": xt_a, "xt_b": xl, "w1s": w1s, "w2t": w2t,
             "txt_a": txt_a, "txt_b": tl, "yext": yext}
        )
    return in_maps


def kernel(x, train_X, Y, W1, W2, h, *, _trace=False):
    x = np.asarray(x, dtype=np.float32)
    train_X = np.asarray(train_X, dtype=np.float32)
    Y = np.asarray(Y, dtype=np.float32)
    W1 = np.asarray(W1, dtype=np.float32)
    W2 = np.asarray(W2, dtype=np.float32)
    h = np.asarray(h, dtype=np.float32)

    c = float(-0.5 / (h[0] ** 2))
    key = (c,)
    if key not in _cache:
        _cache[key] = _build(c)
    nc = _cache[key]

    in_maps = _prep_inputs(x, train_X, Y, W1, W2)
    res = run_bass_kernel_spmd(
        nc, in_maps, core_ids=list(range(N_CORES)), trace=_trace
    )
    parts = np.stack([r["out"] for r in res.results])  # [8, 20, 512]
    tot = parts.sum(axis=0, dtype=np.float64)
    final = np.ascontiguousarray(
        (tot[:DOUT] / tot[DOUT:]).T.astype(np.float32)
    )
    if _trace:
        return final, res
    return final


# revision 32
# speedup vs baseline: 1.0379x; 1.0060x over previous
"""Trainium2 Bass kernel for Nadaraya-Watson kernel regression over MLP embeddings.

Computes: out[b,d] = sum_n K[n,b,d]*Y[n,d] / sum_n K[n,b,d]
where K = exp(-0.5*((z[n,d]-zw[b,d])/h)^2), z/zw are 2-layer MLP embeddings of
train/query points (ReLU MLP 64->128->10).

Strategy (8 NeuronCores, data-parallel over the train axis N, ~100us/core):
 - shard train_X/Y over 8 cores (1250 rows each, padded to 1280 = 10 tiles
   of 128); pad rows are masked via the ones-column of the Y-side weights.
 - layer-1 embedding operands are hi/lo bf16-split on the host (exact fp32
   products in the fp32 PSUM accumulator at full bf16 PE rate); layer 2 runs
   fp32. Embeddings are bit-accurate to ~1e-5.
 - u^2 = z^2 - 2 z zw + zw^2 is built as a K=7 bf16 matmul per (d, n-tile)
   from hi/lo bf16 splits of z^2, -2z, zw, zw^2 (u^2 accurate to ~1e-4),
   streamed at full PE rate; exp on ScalarE with scale=-0.5/h^2 folded in,
   over 3-PSUM-bank [128,1536] chunks (ScalarE is the ~53us bottleneck).
 - PE array packing: u^2 matmuls 2-way row-tiled (partition offsets 0/64,
   operands replicated there via one contiguous DRAM-bounce DMA per group);
   numerator+denominator reduce matmuls (M=2, contracting n on partitions)
   4-way column-tiled (t%4) into one PSUM accumulator bank per d.
 - the main loop is software-pipelined over flat uniform 3-tile chunks:
   reduce matmuls of chunk k are emitted after the u^2 matmuls + exp of
   chunk k+1 so the in-order PE queue never stalls on ScalarE.
 - each core outputs [20,512] partials (10 numerator rows, 10 denominator
   rows); the host sums the 8 partials, divides, transposes -- the
   data-parallel all-reduce + unshard step of the hint.
"""

import ml_dtypes
import numpy as np

import concourse.bass as bass
import concourse.tile as tile
from concourse import bacc, mybir
from concourse.bass_utils import run_bass_kernel_spmd

F32 = mybir.dt.float32
BF16 = mybir.dt.bfloat16
AF = mybir.ActivationFunctionType

N_CORES = 8
B = 512          # batch (queries)
DOUT = 10
DIN = 64
DHID = 128
N_TRAIN = 10000
NSH = 1280       # padded per-core shard rows
NT = NSH // 128  # 10 n-tiles per core
SHARD = N_TRAIN // N_CORES  # 1250 valid rows per core

_cache = {}


def _build(c: float):
    """Build + compile the per-core Bass kernel. c = -0.5/h^2."""
    nc = bacc.Bacc(
        "TRN2",
        target_bir_lowering=False,
        debug=False,
        enable_asserts=False,
        num_devices=N_CORES,
    )

    # layer-1 operands arrive hi/lo bf16-split from the host:
    #   xt_a/txt_a = [Xhi; Xhi] (dup at partition 64), xt_b/txt_b = Xlo
    #   w1s = [W1hiT; W1loT]
    xta_d = nc.dram_tensor("xt_a", [2 * DIN, B], BF16, kind="ExternalInput")
    xtb_d = nc.dram_tensor("xt_b", [DIN, B], BF16, kind="ExternalInput")
    txta_d = nc.dram_tensor("txt_a", [2 * DIN, NSH], BF16, kind="ExternalInput")
    txtb_d = nc.dram_tensor("txt_b", [DIN, NSH], BF16, kind="ExternalInput")
    w1s_d = nc.dram_tensor("w1s", [2 * DIN, DHID], BF16, kind="ExternalInput")
    w2t_d = nc.dram_tensor("w2t", [DHID, DOUT], F32, kind="ExternalInput")   # W2^T
    yext_d = nc.dram_tensor("yext", [NSH, 2 * DOUT], F32, kind="ExternalInput")
    out_d = nc.dram_tensor("out", [2 * DOUT, B], F32, kind="ExternalOutput")

    with tile.TileContext(nc) as tc:
        with (
            tc.tile_pool(name="const", bufs=1) as const,
            tc.tile_pool(name="work", bufs=1) as work,
            tc.tile_pool(name="sums", bufs=4) as sums,
            tc.tile_pool(name="psB", bufs=2, space=bass.MemorySpace.PSUM) as psB,
            tc.tile_pool(name="accp", bufs=2, space=bass.MemorySpace.PSUM) as accp,
            tc.tile_pool(name="ksbp", bufs=3) as ksbp,
            tc.tile_pool(name="dram", bufs=1, space="DRAM") as dram,
        ):
            # ---- load inputs (spread across DMA queues) ----
            # W1^T / trainX^T replicated at partitions 0 and 64 for 2-way
            # row-tiled K=64 embedding matmuls.
            sb_txta = const.tile([2 * DIN, NSH], BF16)
            nc.sync.dma_start(sb_txta[:], txta_d.ap())
            sb_txtb = const.tile([DIN, NSH], BF16)
            nc.sync.dma_start(sb_txtb[:], txtb_d.ap())
            sb_w1s = const.tile([2 * DIN, DHID], BF16)
            nc.scalar.dma_start(sb_w1s[:], w1s_d.ap())
            sb_w2t = const.tile([DHID, DOUT], F32)
            nc.scalar.dma_start(sb_w2t[:], w2t_d.ap())
            sb_xta = const.tile([2 * DIN, B], BF16)
            nc.scalar.dma_start(sb_xta[:], xta_d.ap())
            sb_xtb = const.tile([DIN, B], BF16)
            nc.scalar.dma_start(sb_xtb[:], xtb_d.ap())
            # yext [1280, 20] -> [128, 10, 20] (n-within-tile on partitions)
            sb_y32 = const.tile([128, NT, 2 * DOUT], F32)
            nc.gpsimd.dma_start(
                sb_y32[:], yext_d.ap().rearrange("(t p) c -> p t c", p=128)
            )
            sb_yb = const.tile([128, NT, 2 * DOUT], BF16)
            nc.vector.tensor_copy(sb_yb[:], sb_y32[:])

            # ---- embeddings ----
            # PE order: Ht -> Hq -> Zt -> Zq so the (longer) phi chain starts
            # as early as possible and the psi chain overlaps it.
            ps_ht = psB.tile([128, 1536], F32, tag="pu")
            for a, b_ in ((0, 512), (512, 1024), (1024, 1280)):
                nc.tensor.matmul(
                    ps_ht[:, a:b_], sb_w1s[:], sb_txta[:, a:b_],
                    start=True, stop=False,
                )
                nc.tensor.matmul(
                    ps_ht[:, a:b_], sb_w1s[0:DIN, :], sb_txtb[:, a:b_],
                    start=False, stop=True,
                )
            sb_ht = work.tile([DHID, NSH], F32)
            for a, b_ in ((0, 512), (512, 1024), (1024, 1280)):
                nc.scalar.activation(sb_ht[:, a:b_], ps_ht[:, a:b_], AF.Relu)

            # Zt matmuls (M=10) use col tiles 0/32/64 for 3-way concurrency.
            # sb_zt is stored group-major: n' = (g, t5, p) with tile t = 2*t5+g,
            # so the phi scratch -> replica DMAs are fully contiguous per group.
            ps_zt = psB.tile([128, 1536], F32, tag="pu")
            for i, (a, b_) in enumerate(((0, 512), (512, 1024), (1024, 1280))):
                nc.tensor.matmul(
                    ps_zt[32 * i:32 * i + DOUT, a:b_], sb_w2t[:], sb_ht[:, a:b_],
                    start=True, stop=True, tile_position=(0, 32 * i),
                )
            ps_hq = psB.tile([128, 1536], F32, tag="pu")
            nc.tensor.matmul(
                ps_hq[:, :B], sb_w1s[:], sb_xta[:], start=True, stop=False
            )
            nc.tensor.matmul(
                ps_hq[:, :B], sb_w1s[0:DIN, :], sb_xtb[:], start=False, stop=True
            )
            sb_hq = work.tile([DHID, B], F32)
            nc.vector.tensor_relu(sb_hq[:], ps_hq[:, :B])
            ps_zq = psB.tile([128, 1536], F32, tag="pu")
            nc.tensor.matmul(
                ps_zq[0:DOUT, :B], sb_w2t[:], sb_hq[:], start=True, stop=True
            )

            # sb_zt is stored t%4-group-major with tiles-per-group padded to
            # 3 (slots (g,2) for g>=2 are dead): n' = (g4, t4, p), t = 4*t4+g4.
            sb_zt2 = work.tile([DOUT, 4, 3, 128], F32)
            nc.vector.tensor_copy(sb_zt2[:, :, 0, :], ps_zt[0:DOUT, 0:512])
            nc.scalar.activation(
                sb_zt2[:, :, 1, :], ps_zt[32:32 + DOUT, 512:1024], AF.Copy
            )
            nc.scalar.activation(
                sb_zt2[:, 0:2, 2, :], ps_zt[64:64 + DOUT, 1024:1280], AF.Copy
            )
            sb_zt = sb_zt2[:].rearrange("d g t p -> d (g t p)")
            sb_zq = work.tile([DOUT, B], F32)
            nc.vector.tensor_copy(sb_zq[:], ps_zq[0:DOUT, :B])

            # ---- phi pieces (train side), [10, 1280] ops ----
            # u^2[n,b] = sh*1 + sl*1 + 1*qh + 1*ql + m2zh*wh + m2zh*wl + m2zl*wh
            # staging rows phi: 0 sh, 1 sl, 2 ones, 3 ones, 4 m2zh, 5 m2zh, 6 m2zl
            stg_phi = work.tile([DOUT, 7, 1536], BF16)
            # All ops split into n'-halves (= row-tile group pairs): the first
            # half finishes earlier so its scratch bounce + replica DMAs
            # overlap the second half of the chain. Dead pad slots carry
            # garbage that no matmul ever reads.
            t_sq = work.tile([DOUT, 1536], F32)
            for h0, h1 in ((0, 768), (768, 1536)):
                # m2zh = bf16(-2 z) via ACT scale; m2zl fused on DVE
                nc.scalar.activation(
                    stg_phi[:, 4, h0:h1], sb_zt[:, h0:h1], AF.Copy, scale=-2.0
                )
                nc.vector.scalar_tensor_tensor(
                    stg_phi[:, 6, h0:h1], sb_zt[:, h0:h1], -2.0,
                    stg_phi[:, 4, h0:h1],
                    op0=mybir.AluOpType.mult, op1=mybir.AluOpType.subtract,
                )
                # sh = bf16(z^2) on ACT in parallel with exact fp32 square on DVE
                nc.vector.tensor_mul(t_sq[:, h0:h1], sb_zt[:, h0:h1], sb_zt[:, h0:h1])
                nc.scalar.activation(stg_phi[:, 0, h0:h1], sb_zt[:, h0:h1], AF.Square)
                nc.vector.tensor_sub(
                    stg_phi[:, 1, h0:h1], t_sq[:, h0:h1], stg_phi[:, 0, h0:h1]
                )
            nc.gpsimd.memset(stg_phi[:, 2:4, :], 1.0)  # ones rows

            # ---- psi pieces (query side) ----
            # psi staging rows: 0 ones, 1 ones, 2 qh, 3 ql, 4 wh, 5 wl, 6 wh
            stg_psi = work.tile([DOUT, 7, B], BF16)
            nc.scalar.activation(stg_psi[:, 4, :], sb_zq[:], AF.Copy)
            nc.vector.tensor_sub(stg_psi[:, 5, :], sb_zq[:], stg_psi[:, 4, :])
            t_qw = work.tile([DOUT, B], F32)
            nc.vector.tensor_mul(t_qw[:], sb_zq[:], sb_zq[:])
            nc.scalar.activation(stg_psi[:, 2, :], sb_zq[:], AF.Square)
            nc.vector.tensor_sub(stg_psi[:, 3, :], t_qw[:], stg_psi[:, 2, :])
            nc.gpsimd.memset(stg_psi[:, 0:2, :], 1.0)

            # ---- reorder [d, r, n] -> [r, d, n] via DRAM bounce; replicate at
            # partition offsets 0/32/64/96 for row-tiled u^2 matmuls ----
            # Row-tile group of tile t is g2 = t%2 at partition offsets 0/64.
            # sb_zt's group-major n-order makes scratch and replicas fully
            # contiguous: one bounce + one replica DMA per group. Duplicated
            # rows (phi 5<-4, psi 6<-4) are materialized by the bounce DMAs.
            phic = const.tile([103, DOUT, 3, 128], BF16)
            psic = const.tile([103, DOUT, B], BF16)
            scr_phi = dram.tile([DOUT, 7, 4, 3, 128], BF16)
            scr_psi = dram.tile([7, DOUT, B], BF16)
            scr_v = scr_phi[:].rearrange("d r g t4 p -> d r (g t4 p)")
            for h0, h1 in ((0, 768), (768, 1536)):
                nc.sync.dma_start(scr_v[:, 0:5, h0:h1], stg_phi[:, 0:5, h0:h1])
                nc.sync.dma_start(scr_v[:, 5:6, h0:h1], stg_phi[:, 4:5, h0:h1])
                nc.sync.dma_start(scr_v[:, 6:7, h0:h1], stg_phi[:, 6:7, h0:h1])
            psi_v = scr_psi[:].rearrange("r d b -> d r b")
            nc.scalar.dma_start(psi_v[:, 6:7, :], stg_psi[:, 4:5, :])
            nc.scalar.dma_start(psi_v[:, 0:6, :], stg_psi[:, 0:6, :])
            for g in range(4):
                eng = (nc.scalar, nc.gpsimd, nc.scalar, nc.gpsimd)[g]
                eng.dma_start(psic[32 * g:32 * g + 7], scr_psi[:])
                eng2 = (nc.sync, nc.scalar)[g % 2]
                eng2.dma_start(
                    phic[32 * g:32 * g + 7],
                    scr_phi[:, :, g].rearrange("d r t4 p -> r d t4 p"),
                )

            # ---- software-pipelined main loop ----
            # flat uniform chunks of 3 tiles across the whole (d, t) list --
            # no per-d ragged chunk, so the exp pipeline never hiccups at a
            # d boundary.
            # d0's tiles are reordered so the first two chunks only use row
            # groups {0,1}: their phi replicas depend only on the first half
            # of the piece chain, letting the main loop start while the
            # second half (groups 2/3) is still being staged. Per-(d, gc)
            # emission stays ascending so start/stop flags remain valid.
            d0_order = (0, 1, 4, 5, 8, 9, 2, 3, 6, 7)
            flat = [(0, t) for t in d0_order]
            flat += [(d, t) for d in range(1, DOUT) for t in range(NT)]
            steps = [tuple(flat[i:i + 3]) for i in range(0, len(flat), 3)]
            res2 = work.tile([2, DOUT, B], F32)
            out_v = out_d.ap().rearrange("(two dd) b -> two dd b", two=2)
            kts = {}
            accs = {}
            for k in range(len(steps) + 1):
                if k < len(steps):
                    tt = steps[k]
                    pu = psB.tile([128, 1536], F32, tag="pu")
                    for j, (d, t) in enumerate(tt):
                        g = t % 4
                        nc.tensor.matmul(
                            pu[:, j * B:(j + 1) * B],
                            phic[32 * g:32 * g + 7, d, t // 4, :],
                            psic[32 * g:32 * g + 7, d, :],
                            start=True,
                            stop=True,
                            tile_position=(32 * g, 0),
                        )
                    kt = ksbp.tile([128, 1536], BF16, tag="kt")
                    nc.scalar.activation(
                        kt[:, : len(tt) * B], pu[:, : len(tt) * B], AF.Exp, scale=c
                    )
                    kts[k] = (kt, tt)
                if k > 0:
                    kt, tt = kts.pop(k - 1)
                    for j, (d, t) in enumerate(tt):
                        # column group gc = t%4: group gc accumulates tiles
                        # {gc, gc+4, gc+8}; distinct groups per chunk overlap.
                        if d not in accs:
                            acc_t = accp.tile([98, B], F32, tag="acc")
                            accs[d] = acc_t
                        # final d accumulates in a single group so its tail
                        # drain is one ScalarE copy instead of a DVE add chain
                        gc = 0 if d == DOUT - 1 else t % 4
                        nc.tensor.matmul(
                            accs[d][32 * gc:32 * gc + 2, :],
                            sb_yb[:, t, 2 * d:2 * d + 2],
                            kt[:, j * B:(j + 1) * B],
                            start=(t == 0 if d == DOUT - 1 else t < 4),
                            stop=(t == NT - 1 if d == DOUT - 1 else t >= NT - 4),
                            tile_position=(0, 32 * gc),
                        )
                    d9 = [d for (d, t) in tt if t == NT - 1]
                    if d9:
                        # d complete: sum the 4 column-tile partials
                        # (DVE reads at most one PSUM operand per op)
                        d = d9[0]
                        acc4 = accs.pop(d)
                        if d == DOUT - 1:
                            nc.scalar.activation(
                                res2[:, d, :], acc4[0:2, :], AF.Copy
                            )
                        else:
                            s0 = sums.tile([2, B], F32, tag="s0")
                            nc.vector.tensor_copy(s0[:], acc4[0:2, :])
                            s1 = sums.tile([2, B], F32, tag="s1")
                            nc.vector.tensor_add(s1[:], s0[:], acc4[32:34, :])
                            s2 = sums.tile([2, B], F32, tag="s2")
                            nc.vector.tensor_add(s2[:], s1[:], acc4[64:66, :])
                            nc.vector.tensor_add(
                                res2[:, d, :], s2[:], acc4[96:98, :]
                            )
                        nc.sync.dma_start(out_v[:, d, :], res2[:, d, :])

    nc.compile()
    return nc


def _split_hi_lo(a32):
    hi = a32.astype(ml_dtypes.bfloat16)
    lo = (a32 - hi.astype(np.float32)).astype(ml_dtypes.bfloat16)
    return hi, lo


def _prep_inputs(x, train_X, Y, W1, W2):
    xh, xl = _split_hi_lo(np.ascontiguousarray(x.T, dtype=np.float32))
    xt_a = np.concatenate([xh, xh], axis=0)
    w1h, w1l = _split_hi_lo(np.ascontiguousarray(W1.T, dtype=np.float32))
    w1s = np.concatenate([w1h, w1l], axis=0)
    w2t = np.ascontiguousarray(W2.T, dtype=np.float32)
    in_maps = []
    for i in range(N_CORES):
        sl = slice(i * SHARD, (i + 1) * SHARD)
        txt = np.zeros((DIN, NSH), dtype=np.float32)
        txt[:, :SHARD] = train_X[sl].T
        th, tl = _split_hi_lo(txt)
        txt_a = np.concatenate([th, th], axis=0)
        yext = np.zeros((NSH, 2 * DOUT), dtype=np.float32)
        yext[:SHARD, 0::2] = Y[sl]
        yext[:SHARD, 1::2] = 1.0
        in_maps.append(
            {"xt_a": xt_a, "xt_b": xl, "w1s": w1s, "w2t": w2t,
             "txt_a": txt_a, "txt_b": tl, "yext": yext}
        )
    return in_maps


def kernel(x, train_X, Y, W1, W2, h, *, _trace=False):
    x = np.asarray(x, dtype=np.float32)
    train_X = np.asarray(train_X, dtype=np.float32)
    Y = np.asarray(Y, dtype=np.float32)
    W1 = np.asarray(W1, dtype=np.float32)
    W2 = np.asarray(W2, dtype=np.float32)
    h = np.asarray(h, dtype=np.float32)

    c = float(-0.5 / (h[0] ** 2))
    key = (c,)
    if key not in _cache:
        _cache[key] = _build(c)
    nc = _cache[key]

    in_maps = _prep_inputs(x, train_X, Y, W1, W2)
    res = run_bass_kernel_spmd(
        nc, in_maps, core_ids=list(range(N_CORES)), trace=_trace
    )
    parts = np.stack([r["out"] for r in res.results])  # [8, 20, 512]
    tot = parts.sum(axis=0, dtype=np.float64)
    final = np.ascontiguousarray(
        (tot[:DOUT] / tot[DOUT:]).T.astype(np.float32)
    )
    if _trace:
        return final, res
    return final


# revision 33
# speedup vs baseline: 1.0557x; 1.0172x over previous
"""Trainium2 Bass kernel for Nadaraya-Watson kernel regression over MLP embeddings.

Computes: out[b,d] = sum_n K[n,b,d]*Y[n,d] / sum_n K[n,b,d]
where K = exp(-0.5*((z[n,d]-zw[b,d])/h)^2), z/zw are 2-layer MLP embeddings of
train/query points (ReLU MLP 64->128->10).

Strategy (8 NeuronCores, data-parallel over the train axis N, ~100us/core):
 - shard train_X/Y over 8 cores (1250 rows each, padded to 1280 = 10 tiles
   of 128); pad rows are masked via the ones-column of the Y-side weights.
 - layer-1 embedding operands are hi/lo bf16-split on the host (exact fp32
   products in the fp32 PSUM accumulator at full bf16 PE rate); layer 2 runs
   fp32. Embeddings are bit-accurate to ~1e-5.
 - u^2 = z^2 - 2 z zw + zw^2 is built as a K=7 bf16 matmul per (d, n-tile)
   from hi/lo bf16 splits of z^2, -2z, zw, zw^2 (u^2 accurate to ~1e-4),
   streamed at full PE rate; exp on ScalarE with scale=-0.5/h^2 folded in,
   over 3-PSUM-bank [128,1536] chunks (ScalarE is the ~53us bottleneck).
 - PE array packing: u^2 matmuls 2-way row-tiled (partition offsets 0/64,
   operands replicated there via one contiguous DRAM-bounce DMA per group);
   numerator+denominator reduce matmuls (M=2, contracting n on partitions)
   4-way column-tiled (t%4) into one PSUM accumulator bank per d.
 - the main loop is software-pipelined over flat uniform 3-tile chunks:
   reduce matmuls of chunk k are emitted after the u^2 matmuls + exp of
   chunk k+1 so the in-order PE queue never stalls on ScalarE.
 - each core outputs [20,512] partials (10 numerator rows, 10 denominator
   rows); the host sums the 8 partials, divides, transposes -- the
   data-parallel all-reduce + unshard step of the hint.
"""

import ml_dtypes
import numpy as np

import concourse.bass as bass
import concourse.tile as tile
from concourse import bacc, mybir
from concourse.bass_utils import run_bass_kernel_spmd

F32 = mybir.dt.float32
BF16 = mybir.dt.bfloat16
AF = mybir.ActivationFunctionType

N_CORES = 8
B = 512          # batch (queries)
DOUT = 10
DIN = 64
DHID = 128
N_TRAIN = 10000
NSH = 1280       # padded per-core shard rows
NT = NSH // 128  # 10 n-tiles per core
SHARD = N_TRAIN // N_CORES  # 1250 valid rows per core

_cache = {}


def _build(c: float):
    """Build + compile the per-core Bass kernel. c = -0.5/h^2."""
    nc = bacc.Bacc(
        "TRN2",
        target_bir_lowering=False,
        debug=False,
        enable_asserts=False,
        num_devices=N_CORES,
    )

    # layer-1 operands arrive hi/lo bf16-split from the host:
    #   xt_a/txt_a = [Xhi; Xhi] (dup at partition 64), xt_b/txt_b = Xlo
    #   w1s = [W1hiT; W1loT]
    xta_d = nc.dram_tensor("xt_a", [2 * DIN, B], BF16, kind="ExternalInput")
    xtb_d = nc.dram_tensor("xt_b", [DIN, B], BF16, kind="ExternalInput")
    txta_d = nc.dram_tensor("txt_a", [2 * DIN, NSH], BF16, kind="ExternalInput")
    txtb_d = nc.dram_tensor("txt_b", [DIN, NSH], BF16, kind="ExternalInput")
    w1s_d = nc.dram_tensor("w1s", [2 * DIN, DHID], BF16, kind="ExternalInput")
    w2t_d = nc.dram_tensor("w2t", [DHID, DOUT], F32, kind="ExternalInput")   # W2^T
    yext_d = nc.dram_tensor("yext", [NSH, 2 * DOUT], F32, kind="ExternalInput")
    out_d = nc.dram_tensor("out", [2 * DOUT, B], F32, kind="ExternalOutput")

    with tile.TileContext(nc) as tc:
        with (
            tc.tile_pool(name="const", bufs=1) as const,
            tc.tile_pool(name="work", bufs=1) as work,
            tc.tile_pool(name="sums", bufs=4) as sums,
            tc.tile_pool(name="psB", bufs=2, space=bass.MemorySpace.PSUM) as psB,
            tc.tile_pool(name="accp", bufs=2, space=bass.MemorySpace.PSUM) as accp,
            tc.tile_pool(name="ksbp", bufs=3) as ksbp,
            tc.tile_pool(name="dram", bufs=1, space="DRAM") as dram,
        ):
            # ---- load inputs (spread across DMA queues) ----
            # W1^T / trainX^T replicated at partitions 0 and 64 for 2-way
            # row-tiled K=64 embedding matmuls.
            sb_txta = const.tile([2 * DIN, NSH], BF16)
            nc.sync.dma_start(sb_txta[:], txta_d.ap())
            sb_txtb = const.tile([DIN, NSH], BF16)
            nc.sync.dma_start(sb_txtb[:], txtb_d.ap())
            sb_w1s = const.tile([2 * DIN, DHID], BF16)
            nc.scalar.dma_start(sb_w1s[:], w1s_d.ap())
            sb_w2t = const.tile([DHID, DOUT], F32)
            nc.scalar.dma_start(sb_w2t[:], w2t_d.ap())
            sb_xta = const.tile([2 * DIN, B], BF16)
            nc.scalar.dma_start(sb_xta[:], xta_d.ap())
            sb_xtb = const.tile([DIN, B], BF16)
            nc.scalar.dma_start(sb_xtb[:], xtb_d.ap())
            # yext [1280, 20] -> [128, 10, 20] (n-within-tile on partitions)
            sb_y32 = const.tile([128, NT, 2 * DOUT], F32)
            nc.gpsimd.dma_start(
                sb_y32[:], yext_d.ap().rearrange("(t p) c -> p t c", p=128)
            )
            sb_yb = const.tile([128, NT, 2 * DOUT], BF16)
            nc.vector.tensor_copy(sb_yb[:], sb_y32[:])

            # ---- embeddings ----
            # PE order: Ht -> Hq -> Zt -> Zq so the (longer) phi chain starts
            # as early as possible and the psi chain overlaps it.
            ps_ht = psB.tile([128, 1536], F32, tag="pu")
            for a, b_ in ((0, 512), (512, 1024), (1024, 1280)):
                nc.tensor.matmul(
                    ps_ht[:, a:b_], sb_w1s[:], sb_txta[:, a:b_],
                    start=True, stop=False,
                )
                nc.tensor.matmul(
                    ps_ht[:, a:b_], sb_w1s[0:DIN, :], sb_txtb[:, a:b_],
                    start=False, stop=True,
                )
            sb_ht = work.tile([DHID, NSH], F32)
            for a, b_ in ((0, 512), (512, 1024), (1024, 1280)):
                nc.scalar.activation(sb_ht[:, a:b_], ps_ht[:, a:b_], AF.Relu)

            # Zt matmuls (M=10) use col tiles 0/32/64 for 3-way concurrency.
            # sb_zt is stored group-major: n' = (g, t5, p) with tile t = 2*t5+g,
            # so the phi scratch -> replica DMAs are fully contiguous per group.
            ps_zt = psB.tile([128, 1536], F32, tag="pu")
            for i, (a, b_) in enumerate(((0, 512), (512, 1024), (1024, 1280))):
                nc.tensor.matmul(
                    ps_zt[32 * i:32 * i + DOUT, a:b_], sb_w2t[:], sb_ht[:, a:b_],
                    start=True, stop=True, tile_position=(0, 32 * i),
                )
            ps_hq = psB.tile([128, 1536], F32, tag="pu")
            nc.tensor.matmul(
                ps_hq[:, :B], sb_w1s[:], sb_xta[:], start=True, stop=False
            )
            nc.tensor.matmul(
                ps_hq[:, :B], sb_w1s[0:DIN, :], sb_xtb[:], start=False, stop=True
            )
            sb_hq = work.tile([DHID, B], F32)
            nc.vector.tensor_relu(sb_hq[:], ps_hq[:, :B])
            ps_zq = psB.tile([128, 1536], F32, tag="pu")
            nc.tensor.matmul(
                ps_zq[0:DOUT, :B], sb_w2t[:], sb_hq[:], start=True, stop=True
            )

            # sb_zt is stored t%4-group-major with tiles-per-group padded to
            # 3 (slots (g,2) for g>=2 are dead): n' = (g4, t4, p), t = 4*t4+g4.
            sb_zt2 = work.tile([DOUT, 4, 3, 128], F32)
            nc.vector.tensor_copy(sb_zt2[:, :, 0, :], ps_zt[0:DOUT, 0:512])
            nc.scalar.activation(
                sb_zt2[:, :, 1, :], ps_zt[32:32 + DOUT, 512:1024], AF.Copy
            )
            nc.scalar.activation(
                sb_zt2[:, 0:2, 2, :], ps_zt[64:64 + DOUT, 1024:1280], AF.Copy
            )
            sb_zt = sb_zt2[:].rearrange("d g t p -> d (g t p)")
            sb_zq = work.tile([DOUT, B], F32)
            nc.vector.tensor_copy(sb_zq[:], ps_zq[0:DOUT, :B])

            # ---- phi pieces (train side), [10, 1280] ops ----
            # u^2[n,b] = sh*1 + sl*1 + 1*qh + 1*ql + m2zh*wh + m2zh*wl + m2zl*wh
            # staging rows phi: 0 sh, 1 sl, 2 ones, 3 ones, 4 m2zh, 5 m2zh, 6 m2zl
            stg_phi = work.tile([DOUT, 7, 1536], BF16)
            # All ops split into n'-halves (= row-tile group pairs): the first
            # half finishes earlier so its scratch bounce + replica DMAs
            # overlap the second half of the chain. Dead pad slots carry
            # garbage that no matmul ever reads.
            t_sq = work.tile([DOUT, 1536], F32)
            for h0, h1 in ((0, 768), (768, 1536)):
                # m2zh = bf16(-2 z) via ACT scale; m2zl fused on DVE
                nc.scalar.activation(
                    stg_phi[:, 4, h0:h1], sb_zt[:, h0:h1], AF.Copy, scale=-2.0
                )
                nc.vector.scalar_tensor_tensor(
                    stg_phi[:, 6, h0:h1], sb_zt[:, h0:h1], -2.0,
                    stg_phi[:, 4, h0:h1],
                    op0=mybir.AluOpType.mult, op1=mybir.AluOpType.subtract,
                )
                # sh = bf16(z^2) on ACT in parallel with exact fp32 square on DVE
                nc.vector.tensor_mul(t_sq[:, h0:h1], sb_zt[:, h0:h1], sb_zt[:, h0:h1])
                nc.scalar.activation(stg_phi[:, 0, h0:h1], sb_zt[:, h0:h1], AF.Square)
                nc.vector.tensor_sub(
                    stg_phi[:, 1, h0:h1], t_sq[:, h0:h1], stg_phi[:, 0, h0:h1]
                )
            nc.gpsimd.memset(stg_phi[:, 2:4, :], 1.0)  # ones rows

            # ---- psi pieces (query side) ----
            # psi staging rows: 0 ones, 1 ones, 2 qh, 3 ql, 4 wh, 5 wl, 6 wh
            stg_psi = work.tile([DOUT, 7, B], BF16)
            nc.scalar.activation(stg_psi[:, 4, :], sb_zq[:], AF.Copy)
            nc.vector.tensor_sub(stg_psi[:, 5, :], sb_zq[:], stg_psi[:, 4, :])
            t_qw = work.tile([DOUT, B], F32)
            nc.vector.tensor_mul(t_qw[:], sb_zq[:], sb_zq[:])
            nc.scalar.activation(stg_psi[:, 2, :], sb_zq[:], AF.Square)
            nc.vector.tensor_sub(stg_psi[:, 3, :], t_qw[:], stg_psi[:, 2, :])
            nc.gpsimd.memset(stg_psi[:, 0:2, :], 1.0)

            # ---- reorder [d, r, n] -> [r, d, n] via DRAM bounce; replicate at
            # partition offsets 0/32/64/96 for row-tiled u^2 matmuls ----
            # Row-tile group of tile t is g2 = t%2 at partition offsets 0/64.
            # sb_zt's group-major n-order makes scratch and replicas fully
            # contiguous: one bounce + one replica DMA per group. Duplicated
            # rows (phi 5<-4, psi 6<-4) are materialized by the bounce DMAs.
            phic = const.tile([103, DOUT, 3, 128], BF16)
            psic = const.tile([103, DOUT, B], BF16)
            scr_phi = dram.tile([DOUT, 7, 4, 3, 128], BF16)
            scr_psi = dram.tile([7, DOUT, B], BF16)
            scr_v = scr_phi[:].rearrange("d r g t4 p -> d r (g t4 p)")
            for h0, h1 in ((0, 768), (768, 1536)):
                nc.sync.dma_start(scr_v[:, 0:5, h0:h1], stg_phi[:, 0:5, h0:h1])
                nc.sync.dma_start(scr_v[:, 5:6, h0:h1], stg_phi[:, 4:5, h0:h1])
                nc.sync.dma_start(scr_v[:, 6:7, h0:h1], stg_phi[:, 6:7, h0:h1])
            psi_v = scr_psi[:].rearrange("r d b -> d r b")
            nc.scalar.dma_start(psi_v[:, 6:7, :], stg_psi[:, 4:5, :])
            nc.scalar.dma_start(psi_v[:, 0:6, :], stg_psi[:, 0:6, :])
            for g in range(4):
                eng = (nc.scalar, nc.gpsimd, nc.scalar, nc.gpsimd)[g]
                eng.dma_start(psic[32 * g:32 * g + 7], scr_psi[:])
                eng2 = (nc.sync, nc.scalar)[g % 2]
                eng2.dma_start(
                    phic[32 * g:32 * g + 7],
                    scr_phi[:, :, g].rearrange("d r t4 p -> r d t4 p"),
                )

            # ---- software-pipelined main loop ----
            # flat uniform chunks of 3 tiles across the whole (d, t) list --
            # no per-d ragged chunk, so the exp pipeline never hiccups at a
            # d boundary.
            # d0's tiles are reordered so the first two chunks only use row
            # groups {0,1}: their phi replicas depend only on the first half
            # of the piece chain, letting the main loop start while the
            # second half (groups 2/3) is still being staged. Per-(d, gc)
            # emission stays ascending so start/stop flags remain valid.
            d0_order = (0, 1, 4, 5, 8, 9, 2, 3, 6, 7)
            flat = [(0, t) for t in d0_order]
            flat += [(d, t) for d in range(1, DOUT) for t in range(NT)]
            steps = [tuple(flat[i:i + 3]) for i in range(0, len(flat), 3)]
            last_t = {}
            for d_, t_ in flat:
                last_t[d_] = t_
            res2 = work.tile([2, DOUT, B], F32)
            out_v = out_d.ap().rearrange("(two dd) b -> two dd b", two=2)
            kts = {}
            accs = {}
            for k in range(len(steps) + 1):
                if k < len(steps):
                    tt = steps[k]
                    pu = psB.tile([128, 1536], F32, tag="pu")
                    for j, (d, t) in enumerate(tt):
                        g = t % 4
                        nc.tensor.matmul(
                            pu[:, j * B:(j + 1) * B],
                            phic[32 * g:32 * g + 7, d, t // 4, :],
                            psic[32 * g:32 * g + 7, d, :],
                            start=True,
                            stop=True,
                            tile_position=(32 * g, 0),
                        )
                    kt = ksbp.tile([128, 1536], BF16, tag="kt")
                    nc.scalar.activation(
                        kt[:, : len(tt) * B], pu[:, : len(tt) * B], AF.Exp, scale=c
                    )
                    kts[k] = (kt, tt)
                if k > 0:
                    kt, tt = kts.pop(k - 1)
                    for j, (d, t) in enumerate(tt):
                        # column group gc = t%4: group gc accumulates tiles
                        # {gc, gc+4, gc+8}; distinct groups per chunk overlap.
                        if d not in accs:
                            acc_t = accp.tile([98, B], F32, tag="acc")
                            accs[d] = acc_t
                        # final d accumulates in a single group so its tail
                        # drain is one ScalarE copy instead of a DVE add chain
                        gc = 0 if d == DOUT - 1 else t % 4
                        nc.tensor.matmul(
                            accs[d][32 * gc:32 * gc + 2, :],
                            sb_yb[:, t, 2 * d:2 * d + 2],
                            kt[:, j * B:(j + 1) * B],
                            start=(t == 0 if d == DOUT - 1 else t < 4),
                            stop=(t == NT - 1 if d == DOUT - 1 else t >= NT - 4),
                            tile_position=(0, 32 * gc),
                        )
                    d9 = [d for (d, t) in tt if t == last_t[d]]
                    if d9:
                        # d complete: sum the 4 column-tile partials
                        # (DVE reads at most one PSUM operand per op)
                        d = d9[0]
                        acc4 = accs.pop(d)
                        if d == DOUT - 1:
                            nc.scalar.activation(
                                res2[:, d, :], acc4[0:2, :], AF.Copy
                            )
                        else:
                            s0 = sums.tile([2, B], F32, tag="s0")
                            nc.vector.tensor_copy(s0[:], acc4[0:2, :])
                            s1 = sums.tile([2, B], F32, tag="s1")
                            nc.vector.tensor_add(s1[:], s0[:], acc4[32:34, :])
                            s2 = sums.tile([2, B], F32, tag="s2")
                            nc.vector.tensor_add(s2[:], s1[:], acc4[64:66, :])
                            nc.vector.tensor_add(
                                res2[:, d, :], s2[:], acc4[96:98, :]
                            )
                        nc.sync.dma_start(out_v[:, d, :], res2[:, d, :])

    nc.compile()
    return nc


def _split_hi_lo(a32):
    hi = a32.astype(ml_dtypes.bfloat16)
    lo = (a32 - hi.astype(np.float32)).astype(ml_dtypes.bfloat16)
    return hi, lo


def _prep_inputs(x, train_X, Y, W1, W2):
    xh, xl = _split_hi_lo(np.ascontiguousarray(x.T, dtype=np.float32))
    xt_a = np.concatenate([xh, xh], axis=0)
    w1h, w1l = _split_hi_lo(np.ascontiguousarray(W1.T, dtype=np.float32))
    w1s = np.concatenate([w1h, w1l], axis=0)
    w2t = np.ascontiguousarray(W2.T, dtype=np.float32)
    in_maps = []
    for i in range(N_CORES):
        sl = slice(i * SHARD, (i + 1) * SHARD)
        txt = np.zeros((DIN, NSH), dtype=np.float32)
        txt[:, :SHARD] = train_X[sl].T
        th, tl = _split_hi_lo(txt)
        txt_a = np.concatenate([th, th], axis=0)
        yext = np.zeros((NSH, 2 * DOUT), dtype=np.float32)
        yext[:SHARD, 0::2] = Y[sl]
        yext[:SHARD, 1::2] = 1.0
        in_maps.append(
            {"xt_a": xt_a, "xt_b": xl, "w1s": w1s, "w2t": w2t,
             "txt_a": txt_a, "txt_b": tl, "yext": yext}
        )
    return in_maps


def kernel(x, train_X, Y, W1, W2, h, *, _trace=False):
    x = np.asarray(x, dtype=np.float32)
    train_X = np.asarray(train_X, dtype=np.float32)
    Y = np.asarray(Y, dtype=np.float32)
    W1 = np.asarray(W1, dtype=np.float32)
    W2 = np.asarray(W2, dtype=np.float32)
    h = np.asarray(h, dtype=np.float32)

    c = float(-0.5 / (h[0] ** 2))
    key = (c,)
    if key not in _cache:
        _cache[key] = _build(c)
    nc = _cache[key]

    in_maps = _prep_inputs(x, train_X, Y, W1, W2)
    res = run_bass_kernel_spmd(
        nc, in_maps, core_ids=list(range(N_CORES)), trace=_trace
    )
    parts = np.stack([r["out"] for r in res.results])  # [8, 20, 512]
    tot = parts.sum(axis=0, dtype=np.float64)
    final = np.ascontiguousarray(
        (tot[:DOUT] / tot[DOUT:]).T.astype(np.float32)
    )
    if _trace:
        return final, res
    return final
